# revision 1
# baseline (speedup 1.0000x reference)
"""Trainium2 Bass kernel for nn_NeuralRenderer (image_size=256, F=640 faces).

Strategy (per sharding hint): pixel rows sharded across 8 NeuronCores; faces /
textures replicated. Each core rasterizes its 8192-pixel band against all
faces:

  phase 1 (device): per-(pixel,face) barycentric planes w0,w1,w2 and depth are
    affine in (px,py,1) -> computed as a K=3 fp32 matmul on the PE into PSUM.
    ScalarE turns them into relu(-w) penalties (bf16), DVE folds them into a
    packed key  keyn = -depth - 1e34*penalty  and finds the per-pixel argmax
    over faces with the DVE max8/max_index ops (= nearest visible face).
  phase 2 (device): winner's face record (vertex/edge/det/z data, 48B) is
    gathered with indirect DMA; barycentrics are recomputed with the exact
    f32 operation order of the reference (division via bit-exact reciprocal),
    validity uses exact sign tests; texel rows (12B) are gathered from the
    pre-lit tanh'd texture table by indirect DMA; shading + masking on DVE.

Host does only the O(F) per-face setup (projection, affine coefficients,
texture table prep) plus input sharding / output concat.
"""

import numpy as np

IMG = 256
F = 640
NCORES = 8
PPC = IMG * IMG // NCORES    # pixels per core = 8192
NT = PPC // 128              # pixel tiles per core = 64
NREC = 12                    # face record floats
TEXROWS = F * 216
USE_FP32R = True             # PE fp32 "replicated" fast path (1 cyc/col)

_CACHE: dict = {}


# ----------------------------------------------------------------------------
# Device program
# ----------------------------------------------------------------------------

def _build_program(debug=False):
    import concourse.bass as bass
    import concourse.bacc as bacc
    import concourse.mybir as mybir
    import concourse.tile as tile

    dt = mybir.dt
    Alu = mybir.AluOpType
    Act = mybir.ActivationFunctionType

    nc = bacc.Bacc(None, target_bir_lowering=False)
    dbg = {}
    if debug:
        dbg["psA"] = nc.dram_tensor("dbg_psA", [128, 1280], dt.float32, kind="ExternalOutput")
        dbg["keyn"] = nc.dram_tensor("dbg_keyn", [128, 640], dt.float32, kind="ExternalOutput")
        dbg["m8"] = nc.dram_tensor("dbg_m8", [128, NT, 8], dt.float32, kind="ExternalOutput")
        dbg["i8"] = nc.dram_tensor("dbg_i8", [128, NT, 8], dt.uint32, kind="ExternalOutput")
        dbg["crec"] = nc.dram_tensor("dbg_crec", [128, NT, NREC], dt.float32, kind="ExternalOutput")
        dbg["vm"] = nc.dram_tensor("dbg_vm", [128, NT], dt.float32, kind="ExternalOutput")
        dbg["flat"] = nc.dram_tensor("dbg_flat", [128, NT], dt.int32, kind="ExternalOutput")
        dbg["ctex"] = nc.dram_tensor("dbg_ctex", [128, NT, 3], dt.float32, kind="ExternalOutput")
        dbg["b0"] = nc.dram_tensor("dbg_b0", [128, NT], dt.float32, kind="ExternalOutput")
        dbg["echo"] = nc.dram_tensor("dbg_echo", [128, NT], dt.float32, kind="ExternalOutput")
    pxT_d = nc.dram_tensor("pxT", [3, PPC], dt.float16, kind="ExternalInput")
    pxy_d = nc.dram_tensor("pxy", [2, 128, NT], dt.float32, kind="ExternalInput")
    faceBh_d = nc.dram_tensor("faceBh", [3, 4 * F], dt.float16, kind="ExternalInput")
    faceBl_d = nc.dram_tensor("faceBl", [3, 4 * F], dt.float16, kind="ExternalInput")
    frec_d = nc.dram_tensor("frec", [F, NREC], dt.float32, kind="ExternalInput")
    texlit_d = nc.dram_tensor("texlit", [TEXROWS, 3], dt.float32, kind="ExternalInput")
    img_d = nc.dram_tensor("img", [3, 128, NT], dt.float32, kind="ExternalOutput")

    # matmul output segments within a [128, 1280] (2-plane) PSUM tile, each
    # inside a single 512-f32 PSUM bank (bank-aligned, 3 matmuls per half)
    segs = [(0, 512), (512, 1024), (1024, 1280)]

    with tile.TileContext(nc) as tc:
        with (
            tc.tile_pool(name="const", bufs=1) as cp,
            tc.tile_pool(name="work", bufs=4) as wp,
            tc.tile_pool(name="p2", bufs=1) as p2,
            tc.tile_pool(name="psA", bufs=3, space="PSUM") as ppA,
            tc.tile_pool(name="psB", bufs=2, space="PSUM") as ppB,
        ):
            pxT = cp.tile([3, PPC], dt.float16)
            nc.sync.dma_start(pxT[:], pxT_d[:])
            faceBh = cp.tile([3, 4 * F], dt.float16)
            nc.sync.dma_start(faceBh[:], faceBh_d[:])
            faceBl = cp.tile([3, 4 * F], dt.float16)
            nc.sync.dma_start(faceBl[:], faceBl_d[:])
            pxv = cp.tile([128, NT], dt.float32)
            nc.sync.dma_start(pxv[:], pxy_d[0])
            pyv = cp.tile([128, NT], dt.float32)
            nc.sync.dma_start(pyv[:], pxy_d[1])
            m8buf = cp.tile([128, NT, 8], dt.float32)
            i8buf = cp.tile([128, NT, 8], dt.uint32)
            crec = cp.tile([128, NT, NREC], dt.float32)

            # ---------------- phase 1: winner face per pixel ----------------
            for t in range(NT):
                lhsT = pxT[:, t * 128:(t + 1) * 128]
                # plane columns [w0|w1|w2|d] (4*640) split into bank-aligned
                # psum tiles: 1024 + 1024 + 512 so the PE can run ahead of
                # the ScalarE drains (3+2 slots in flight)
                T0 = ppA.tile([128, 1024], dt.float32, tag="pA")
                T1 = ppA.tile([128, 1024], dt.float32, tag="pA")
                T2 = ppB.tile([128, 512], dt.float32, tag="pB")
                for ps, base, width in ((T0, 0, 1024), (T1, 1024, 1024), (T2, 2048, 512)):
                    for s in range(0, width, 512):
                        e = min(s + 512, width)
                        nc.tensor.matmul(ps[:, s:e], lhsT,
                                         faceBh[:, base + s:base + e],
                                         start=True, stop=False)
                        nc.tensor.matmul(ps[:, s:e], lhsT,
                                         faceBl[:, base + s:base + e],
                                         start=False, stop=True)

                # relu(-x) of ALL planes (d's relu is the rD penalty term)
                rAll = wp.tile([128, 2560], dt.bfloat16, tag="rAll")
                nc.scalar.activation(rAll[:, 0:1024], T0[:], Act.Relu, scale=-1.0)
                nc.scalar.activation(rAll[:, 1024:2048], T1[:], Act.Relu, scale=-1.0)
                nc.scalar.activation(rAll[:, 2048:2560], T2[:], Act.Relu, scale=-1.0)
                dneg = wp.tile([128, 640], dt.float32, tag="dneg")
                nc.scalar.activation(dneg[:, 0:128], T1[:, 896:1024], Act.Copy, scale=-1.0)
                nc.scalar.activation(dneg[:, 128:640], T2[:], Act.Copy, scale=-1.0)

                pen0 = wp.tile([128, 640], dt.bfloat16, tag="pen0")
                nc.vector.tensor_tensor(pen0[:], rAll[:, 0:640], rAll[:, 640:1280], op=Alu.add)
                pen1 = wp.tile([128, 640], dt.bfloat16, tag="pen1")
                nc.gpsimd.tensor_tensor(pen1[:], rAll[:, 1280:1920], rAll[:, 1920:2560], op=Alu.add)
                pen2 = wp.tile([128, 640], dt.bfloat16, tag="pen2")
                nc.vector.tensor_tensor(pen2[:], pen0[:], pen1[:], op=Alu.add)
                keyn = wp.tile([128, 640], dt.float32, tag="keyn")
                nc.vector.scalar_tensor_tensor(
                    keyn[:], pen2[:], -1e34, dneg[:],
                    op0=Alu.mult, op1=Alu.add)

                nc.vector.max(m8buf[:, t], keyn[:])
                nc.vector.max_index(i8buf[:, t], m8buf[:, t], keyn[:])
                # winner record gather (one row per partition — the only
                # indirect-DMA shape that matches hardware semantics);
                # issued per tile so it overlaps the raster loop
                nc.gpsimd.indirect_dma_start(
                    out=crec[:, t], out_offset=None, in_=frec_d[:],
                    in_offset=bass.IndirectOffsetOnAxis(
                        ap=i8buf[:, t, 0:1], axis=0))
                if debug and t == 0:
                    psAc = wp.tile([128, 1024], dt.float32, tag="psAc")
                    nc.scalar.activation(psAc[:], T0[:], Act.Copy)
                    nc.sync.dma_start(dbg["psA"][:, 0:1024], psAc[:])
                    nc.sync.dma_start(dbg["keyn"][:], keyn[:])

            # ---------------- phase 2: exact recompute + shading ------------
            def tt(name, in0, in1, op, dtype=dt.float32):
                o = p2.tile([128, NT], dtype, tag=name)
                nc.vector.tensor_tensor(o[:], in0, in1, op=op)
                return o

            def ts(name, in0, s1, s2, op0, op1=None, dtype=dt.float32):
                o = p2.tile([128, NT], dtype, tag=name)
                if op1 is None:
                    nc.vector.tensor_scalar(o[:], in0, s1, None, op0=op0)
                else:
                    nc.vector.tensor_scalar(o[:], in0, s1, s2, op0=op0, op1=op1)
                return o

            ch = lambda k: crec[:, :, k]
            # record: 0:x0 1:y0 2:d0 3:d1 4:e0 5:e1 6:det_s 7:detok
            #         8:z0 9:z1 10:z2 11:texbase(=216*face)

            qx = tt("qx", pxv[:], ch(0), Alu.subtract)
            qy = tt("qy", pyv[:], ch(1), Alu.subtract)
            t1 = tt("t1", qx[:], ch(5), Alu.mult)
            t2 = tt("t2", qy[:], ch(4), Alu.mult)
            n1 = tt("n1", t1[:], t2[:], Alu.subtract)
            t3 = tt("t3", ch(2), qy[:], Alu.mult)
            t4 = tt("t4", ch(3), qx[:], Alu.mult)
            n2 = tt("n2", t3[:], t4[:], Alu.subtract)
            rdet = p2.tile([128, NT], dt.float32)
            nc.vector.reciprocal(rdet[:], ch(6))
            b1 = tt("b1", n1[:], rdet[:], Alu.mult)
            b2 = tt("b2", n2[:], rdet[:], Alu.mult)
            u = ts("u", b1[:], -1.0, 1.0, Alu.mult, Alu.add)       # 1 - b1
            b0 = tt("b0", u[:], b2[:], Alu.subtract)

            # validity: exact sign tests for b1,b2 (sign(n/det) == sign(n*det))
            s1v = tt("s1v", n1[:], ch(6), Alu.mult)
            g1 = ts("g1", s1v[:], 0.0, None, Alu.is_ge)
            s2v = tt("s2v", n2[:], ch(6), Alu.mult)
            g2 = ts("g2", s2v[:], 0.0, None, Alu.is_ge)
            g0 = ts("g0", b0[:], 0.0, None, Alu.is_ge)
            m0 = tt("m0", b0[:], ch(8), Alu.mult)
            m1 = tt("m1", b1[:], ch(9), Alu.mult)
            s01 = tt("s01", m0[:], m1[:], Alu.add)
            m2 = tt("m2", b2[:], ch(10), Alu.mult)
            dw = tt("dw", s01[:], m2[:], Alu.add)
            gd = ts("gd", dw[:], 0.0, None, Alu.is_gt)
            vm = tt("vm", g1[:], g2[:], Alu.mult)
            vm = tt("vmb", vm[:], g0[:], Alu.mult)
            vm = tt("vmc", vm[:], gd[:], Alu.mult)
            vm = tt("vmd", vm[:], ch(7), Alu.mult)

            # texture cell indices: floor(clip(6*b, 0, 5.5)) robust to the
            # hardware's f32->int rounding mode (round-trip + correction)
            def floor_idx(name, b):
                x = ts(name + "x", b[:], 6.0, 0.0, Alu.mult, Alu.max)
                x = ts(name + "c", x[:], 5.5, None, Alu.min)
                ji = p2.tile([128, NT], dt.int32, tag=name + "i")
                nc.vector.tensor_copy(ji[:], x[:])
                jf = p2.tile([128, NT], dt.float32, tag=name + "f")
                nc.vector.tensor_copy(jf[:], ji[:])
                gt = tt(name + "g", jf[:], x[:], Alu.is_gt)
                return tt(name + "r", jf[:], gt[:], Alu.subtract)

            i0f = floor_idx("i0", b0)
            i1f = floor_idx("i1", b1)
            i2f = floor_idx("i2", b2)
            ffb = p2.tile([128, NT], dt.float32)
            nc.vector.scalar_tensor_tensor(ffb[:], i0f[:], 36.0, ch(11),
                                           op0=Alu.mult, op1=Alu.add)
            ffc = p2.tile([128, NT], dt.float32)
            nc.vector.scalar_tensor_tensor(ffc[:], i1f[:], 6.0, ffb[:],
                                           op0=Alu.mult, op1=Alu.add)
            ffd = tt("ffd", ffc[:], i2f[:], Alu.add)
            flat = p2.tile([128, NT], dt.int32)
            nc.vector.tensor_copy(flat[:], ffd[:])

            ctex = p2.tile([128, NT, 3], dt.float32)
            for t in range(NT):
                nc.gpsimd.indirect_dma_start(
                    out=ctex[:, t], out_offset=None, in_=texlit_d[:],
                    in_offset=bass.IndirectOffsetOnAxis(
                        ap=flat[:, t:t + 1], axis=0))

            for c in range(3):
                outp = p2.tile([128, NT], dt.float32, tag=f"outp{c}")
                nc.vector.tensor_tensor(outp[:], ctex[:, :, c], vm[:], op=Alu.mult)
                nc.sync.dma_start(img_d[c], outp[:])

            if debug:
                nc.sync.dma_start(dbg["m8"][:], m8buf[:])
                nc.sync.dma_start(dbg["i8"][:], i8buf[:])
                nc.sync.dma_start(dbg["crec"][:], crec[:])
                nc.sync.dma_start(dbg["vm"][:], vm[:])
                nc.sync.dma_start(dbg["flat"][:], flat[:])
                nc.sync.dma_start(dbg["ctex"][:], ctex[:])
                nc.sync.dma_start(dbg["b0"][:], b0[:])
                nc.sync.dma_start(dbg["echo"][:], pxv[:])

    nc.compile()
    return nc


def _get_program(debug=False):
    key = ("nc", debug)
    if key not in _CACHE:
        _CACHE[key] = _build_program(debug)
    return _CACHE[key]


# ----------------------------------------------------------------------------
# Host-side per-face setup (O(F) work, mirrors reference f32 op order)
# ----------------------------------------------------------------------------

def _host_prep(vertices, faces, textures):
    f32 = np.float32
    v = np.asarray(vertices[0], f32)                        # [N,3]
    f = np.asarray(faces[0]).astype(np.int64)               # [F,3]
    fv = v[f]                                               # [F,3,3]

    # lighting (ambient 0.5 + directional 0.5 * relu(n.[0,0,1]))
    n = np.cross(fv[:, 1] - fv[:, 0], fv[:, 2] - fv[:, 0]).astype(f32)
    nrm = np.linalg.norm(n, axis=-1, keepdims=True).astype(f32)
    n = (n / (nrm + f32(1e-8))).astype(f32)
    light = (f32(0.5) + f32(0.5) * np.maximum(n[:, 2], f32(0.0))).astype(f32)

    # 'look' camera: R == I for these constants; eye (0,0,-2)
    vc = (fv - np.array([0.0, 0.0, -2.0], f32)).astype(f32)
    zc = vc[..., 2].astype(f32)                             # [F,3]
    wfov = f32(np.tan(np.deg2rad(f32(45.0), dtype=f32), dtype=f32))
    xy = (vc[..., :2] / (zc[..., None] * wfov + f32(1e-8))).astype(f32)

    v0 = xy[:, 0]
    dd = (xy[:, 1] - v0).astype(f32)                        # [F,2]
    ee = (xy[:, 2] - v0).astype(f32)                        # [F,2]
    det = (dd[:, 0] * ee[:, 1] - dd[:, 1] * ee[:, 0]).astype(f32)
    det_ok = np.abs(det) > f32(1e-8)
    det_s = np.where(det_ok, det, f32(1.0)).astype(f32)

    # affine coefficients (f64 for accuracy, cast f32)
    x0 = v0[:, 0].astype(np.float64); y0 = v0[:, 1].astype(np.float64)
    d0 = dd[:, 0].astype(np.float64); d1 = dd[:, 1].astype(np.float64)
    e0 = ee[:, 0].astype(np.float64); e1 = ee[:, 1].astype(np.float64)
    ds = det_s.astype(np.float64)
    a1 = np.stack([e1, -e0, e0 * y0 - e1 * x0], -1) / ds[:, None]
    a2 = np.stack([-d1, d0, d1 * x0 - d0 * y0], -1) / ds[:, None]
    a0 = -a1 - a2
    a0[:, 2] += 1.0
    zc64 = zc.astype(np.float64)
    ad = a0 * zc64[:, 0:1] + a1 * zc64[:, 1:2] + a2 * zc64[:, 2:3]

    bad = (~det_ok | ~np.isfinite(a0).all(1) | ~np.isfinite(a1).all(1)
           | ~np.isfinite(a2).all(1) | ~np.isfinite(ad).all(1))
    for a in (a0, a1, a2):
        a[bad] = np.array([0.0, 0.0, -1.0])
    ad[bad] = np.array([0.0, 0.0, 1.0])

    faceB = np.concatenate(
        [a0.T, a1.T, a2.T, ad.T], axis=1).astype(f32)       # [3, 4F]
    faceBh = faceB.astype(np.float16)
    faceBl = (faceB - faceBh.astype(f32)).astype(np.float16)

    frec = np.stack([
        v0[:, 0], v0[:, 1], dd[:, 0], dd[:, 1], ee[:, 0], ee[:, 1],
        det_s, det_ok.astype(f32), zc[:, 0], zc[:, 1], zc[:, 2],
        (np.arange(F) * 216).astype(f32)], -1).astype(f32)  # [F, 12]

    tex = np.tanh(np.asarray(textures[0], f32)).astype(f32)     # [F,6,6,6,3]
    texlit = (tex * light[:, None, None, None, None]).astype(f32)
    texlit = texlit.reshape(TEXROWS, 3)

    return faceBh, faceBl, frec, texlit


def _pixel_buffers():
    f32 = np.float32
    ps = ((np.arange(IMG, dtype=f32) + f32(0.5)) / f32(IMG) * f32(2.0)
          - f32(1.0))
    nps = (-ps).astype(f32)
    j = np.arange(128)
    t = np.arange(NT)
    bufs = []
    for c in range(NCORES):
        g = c * PPC + j[:, None] * NT + t[None, :]          # [128, NT]
        px = ps[g % IMG].astype(f32)
        py = nps[g // IMG].astype(f32)
        pxy = np.stack([px, py]).astype(f32)                # [2, 128, NT]
        M = np.empty((3, NT, 128), f32)
        M[0] = px.T
        M[1] = py.T
        M[2] = 1.0
        pxT = M.reshape(3, PPC).astype(np.float16)          # col t*128+j; exact
        bufs.append((pxT, pxy))
    return bufs


# ----------------------------------------------------------------------------
# Entry point
# ----------------------------------------------------------------------------

def _run(inputs, trace=False):
    from concourse.bass_utils import run_bass_kernel_spmd

    faceBh, faceBl, frec, texlit = _host_prep(
        np.asarray(inputs["vertices"]),
        np.asarray(inputs["faces"]),
        np.asarray(inputs["textures"]))
    nc = _get_program()
    in_maps = []
    for (pxT, pxy) in _pixel_buffers():
        in_maps.append({
            "pxT": pxT, "pxy": pxy, "faceBh": faceBh, "faceBl": faceBl,
            "frec": frec, "texlit": texlit,
        })
    res = run_bass_kernel_spmd(nc, in_maps, list(range(NCORES)), trace=trace)
    outs = [np.asarray(res.results[c]["img"]).reshape(3, PPC)
            for c in range(NCORES)]
    full = np.concatenate(outs, axis=1).reshape(3, IMG, IMG)[None]
    return full.astype(np.float32), res


def kernel(**inputs) -> np.ndarray:
    out, _ = _run(inputs, trace=False)
    return out



# revision 3
# speedup vs baseline: 1.2488x; 1.2488x over previous
"""Trainium2 Bass kernel v2 for nn_NeuralRenderer (256x256, F=640).

Scene-specialized tile raster:
  - image split into 512 blocks of 8x16 px; per block the host culls faces
    by projected bbox into a sorted list, chunked into FT=64-face passes
  - blocks (sorted by list length) are dealt round-robin to the 8 cores so
    every core runs the SAME schedule (K_s passes for slot s) on its own
    data -> one SPMD program, compiled per schedule signature
  - per pass: [3x128]x[3x256] fp16 hi/lo matmul -> psum planes
    [-w0|-w1|-w2|-d]; ScalarE relu(1e34*x) -> penalties (bf16); Pool sums
    plane pairs; PE accumulates -penalty onto the -d psum segment via a
    -identity matmul so the selection key  -d - 1e34*pen  materializes in
    PSUM with no further vector work; DVE copies keys out (two passes per
    copy) and runs one max8+find_index8 per slot over the K*FT keys
  - winner records + texel rows are fetched with batched dma_gather (256B
    rows, int16 indices built via a DMA-transpose shuffle), barycentrics
    recomputed in exact reference f32 op order, texel picked from the
    gathered (face,i0,i1) row by a 6-way predicated select, shaded, stored
"""

import numpy as np

IMG = 256
F = 640
NCORES = 8
NSLOT = 64
FT = 64
NPLANE = 4 * FT              # psum cols per pass
MARGIN = 2e-3
TEXROWS = F * 36             # (face, i0, i1) rows (dma_gather mode)
TEXROWS216 = F * 216         # packed texel rows (indirect mode)
GREC = 64                    # gathered row width (f32) = 256B
GATHER_MODE = "indirect64"   # or "dma_gather"
# indirect mode: spread the many SWDGE gathers; dma_gather: fewer, bigger
NCHUNK = 8 if GATHER_MODE == "indirect64" else 4

_CACHE: dict = {}


# ----------------------------------------------------------------------------
# Device program
# ----------------------------------------------------------------------------

def _build_program(Ks):
    """Ks: per-slot pass counts (len NSLOT, entries may be 0)."""
    import concourse.bass as bass
    import concourse.bacc as bacc
    import concourse.mybir as mybir
    import concourse.tile as tile

    dt = mybir.dt
    Alu = mybir.AluOpType
    Act = mybir.ActivationFunctionType

    npass = int(sum(Ks))
    nrows = npass * FT + 1

    nc = bacc.Bacc(None, target_bir_lowering=False, num_swdge_queues=4)

    pxT_d = nc.dram_tensor("pxT", [3, NSLOT * 128], dt.float16, kind="ExternalInput")
    pxy_d = nc.dram_tensor("pxy", [2, 128, NSLOT], dt.float32, kind="ExternalInput")
    fb_d = nc.dram_tensor("fb", [npass, 3, 2 * NPLANE], dt.float16, kind="ExternalInput")
    negI_d = nc.dram_tensor("negI", [128, 128], dt.bfloat16, kind="ExternalInput")
    sbase_d = nc.dram_tensor("sbase", [128, NSLOT], dt.float32, kind="ExternalInput")
    frec_w = GREC if GATHER_MODE == "dma_gather" else 12
    frec_d = nc.dram_tensor("frec", [nrows, frec_w], dt.float32, kind="ExternalInput")
    if GATHER_MODE == "dma_gather":
        texg_d = nc.dram_tensor("texg", [TEXROWS + 1, GREC], dt.float32, kind="ExternalInput")
    else:
        texg_d = nc.dram_tensor("texg", [TEXROWS216, 3], dt.float32, kind="ExternalInput")
    img_d = nc.dram_tensor("img", [3, 128, NSLOT], dt.float32, kind="ExternalOutput")

    HALF = NSLOT // 2

    with tile.TileContext(nc) as tc:
        with (
            tc.tile_pool(name="const", bufs=1) as cp,
            tc.tile_pool(name="fbp", bufs=6) as fbp,
            tc.tile_pool(name="rp", bufs=4) as rp,
            tc.tile_pool(name="ap", bufs=4) as ap,
            tc.tile_pool(name="keyp", bufs=3) as keyp,
            tc.tile_pool(name="p2", bufs=1) as p2,
            tc.tile_pool(name="ps", bufs=3, space="PSUM") as psp,
        ):
            pxT = cp.tile([3, NSLOT * 128], dt.float16)
            nc.sync.dma_start(pxT[:], pxT_d[:])
            negI = cp.tile([128, 128], dt.bfloat16)
            nc.sync.dma_start(negI[:], negI_d[:])
            pxv = cp.tile([128, NSLOT], dt.float32)
            nc.sync.dma_start(pxv[:], pxy_d[0])
            pyv = cp.tile([128, NSLOT], dt.float32)
            nc.sync.dma_start(pyv[:], pxy_d[1])
            sbase = cp.tile([128, NSLOT], dt.float32)
            nc.sync.dma_start(sbase[:], sbase_d[:])
            m8buf = cp.tile([128, NSLOT, 8], dt.float32)
            i8buf = cp.tile([128, NSLOT, 8], dt.uint32)
            crec = cp.tile([128, NSLOT, GREC], dt.float32)
            ctex = cp.tile([128, NSLOT, GREC], dt.float32)
            vmb = cp.tile([128, NSLOT], dt.float32)
            col3 = cp.tile([128, NSLOT, 3], dt.float32)

            Kmax = max(Ks)
            pidx = 0

            def raster_slot(s):
                nonlocal pidx
                K = Ks[s]
                keyb = keyp.tile([128, Kmax * FT], dt.float32, tag="key")
                lhsT = pxT[:, s * 128:(s + 1) * 128]
                k = 0
                while k < K:
                    pair = 2 if k + 1 < K else 1
                    ps = psp.tile([128, 512], dt.float32, tag="ps")
                    for j in range(pair):
                        p = pidx
                        pidx += 1
                        fb = fbp.tile([3, 2 * NPLANE], dt.float16, tag="fb")
                        nc.sync.dma_start(fb[:], fb_d[p])
                        seg = ps[:, j * NPLANE:(j + 1) * NPLANE]
                        nc.tensor.matmul(seg, lhsT, fb[:, 0:NPLANE],
                                         start=True, stop=False)
                        nc.tensor.matmul(seg, lhsT, fb[:, NPLANE:2 * NPLANE],
                                         start=False, stop=True)
                        r = rp.tile([128, NPLANE], dt.bfloat16, tag="r")
                        nc.scalar.activation(r[:], seg, Act.Relu, scale=1e34)
                        a1 = ap.tile([128, 2 * FT], dt.bfloat16, tag="a1")
                        a1eng = (nc.gpsimd if GATHER_MODE == "dma_gather"
                                 else nc.vector)
                        a1eng.tensor_tensor(
                            a1[:], r[:, 0:2 * FT], r[:, 2 * FT:4 * FT],
                            op=Alu.add)
                        dseg = ps[:, j * NPLANE + 3 * FT:j * NPLANE + 4 * FT]
                        nc.tensor.matmul(dseg, negI[:], a1[:, 0:FT],
                                         start=False, stop=False,
                                         skip_group_check=True)
                        nc.tensor.matmul(dseg, negI[:], a1[:, FT:2 * FT],
                                         start=False, stop=True,
                                         skip_group_check=True)
                    # copy this psum tile's key segment(s) to the key buffer
                    if pair == 2:
                        src = ps[:].rearrange(
                            "p (j x) -> p j x", j=2)[:, :, 3 * FT:4 * FT]
                        dst = keyb[:, k * FT:(k + 2) * FT].rearrange(
                            "p (j x) -> p j x", j=2)
                        nc.vector.tensor_copy(dst, src)
                    else:
                        nc.vector.tensor_copy(
                            keyb[:, k * FT:(k + 1) * FT], ps[:, 3 * FT:4 * FT])
                    k += pair
                nc.vector.max(m8buf[:, s], keyb[:, 0:K * FT])
                nc.vector.max_index(i8buf[:, s], m8buf[:, s], keyb[:, 0:K * FT])

            # ---- batched gather machinery --------------------------------
            def shuffle_idxs(name, rowf, s0, s1):
                """rowf: [128, n] f32 row ids -> int16 idxs [128, n*8]."""
                n = s1 - s0
                r16 = p2.tile([128, 128], dt.int16, tag=f"{name}r16")
                nc.gpsimd.memset(r16[:], 0)
                nc.vector.tensor_copy(r16[:, 0:n], rowf[:])
                T = p2.tile([128, 128], dt.int16, tag=f"{name}T")
                nc.sync.dma_start_transpose(T[:], r16[:])
                Tp = p2.tile([n, 128], dt.int16, tag=f"{name}Tp")
                nc.vector.tensor_copy(
                    Tp[:], T[0:n, :].rearrange("s (q c) -> s q c", c=16).transpose([0, 2, 1]))
                idxs = p2.tile([128, n * 8], dt.int16, tag=f"{name}ix")
                for c in range(16):
                    for rep in range(8):
                        nc.sync.dma_start(
                            idxs[16 * rep + c:16 * rep + c + 1], Tp[:, c * 8:(c + 1) * 8])
                return idxs

            def gather(dst, table_d, idxs, s0, s1, semname):
                n = s1 - s0
                nq = min(4, n)
                per = n // nq
                sem = nc.alloc_semaphore(semname)
                for q in range(nq):
                    nidx = per * 128
                    nc.gpsimd.dma_gather(
                        out_ap=dst[:, s0 + q * per:s0 + (q + 1) * per],
                        in_ap=table_d[:],
                        idxs_ap=idxs[:, q * per * 8:(q + 1) * per * 8],
                        num_idxs=nidx,
                        num_idxs_reg=nidx,
                        elem_size=GREC,
                        queue_num=q,
                    ).then_inc(sem, 16)
                return sem, 16 * nq

            def gather_crec_range(s0, s1):
                h = f"{s0}_{s1}"
                winf = p2.tile([128, s1 - s0], dt.float32, tag=f"winf{h}")
                nc.vector.tensor_copy(winf[:], i8buf[:, s0:s1, 0])
                nc.vector.tensor_tensor(
                    winf[:], winf[:], sbase[:, s0:s1], op=Alu.add)
                if GATHER_MODE == "dma_gather":
                    idxs = shuffle_idxs(f"cr{h}", winf, s0, s1)
                    return gather(crec, frec_d, idxs, s0, s1, f"gcrec{h}")
                rowi = p2.tile([128, s1 - s0], dt.int32, tag=f"rowi{h}")
                nc.vector.tensor_copy(rowi[:], winf[:])
                for s in range(s0, s1):
                    nc.gpsimd.indirect_dma_start(
                        out=crec[:, s, 0:12], out_offset=None,
                        in_=frec_d[:],
                        in_offset=bass.IndirectOffsetOnAxis(
                            ap=rowi[:, s - s0:s - s0 + 1], axis=0))
                return None

            # ---- phase 2: exact recompute + shading ----------------------
            def phase2(rng_, crec_sem):
                s0, s1 = rng_
                h = f"{s0}_{s1}"
                n = s1 - s0
                if crec_sem is not None:
                    sem, val = crec_sem
                    nc.vector.wait_ge(sem, val)

                def tt(name, in0, in1, op, dtype=dt.float32):
                    o = p2.tile([128, n], dtype, tag=f"{name}{h}")
                    nc.vector.tensor_tensor(o[:], in0, in1, op=op)
                    return o

                def ts(name, in0, s1_, s2, op0, op1=None, dtype=dt.float32):
                    o = p2.tile([128, n], dtype, tag=f"{name}{h}")
                    if op1 is None:
                        nc.vector.tensor_scalar(o[:], in0, s1_, None, op0=op0)
                    else:
                        nc.vector.tensor_scalar(o[:], in0, s1_, s2,
                                                op0=op0, op1=op1)
                    return o

                ch = lambda k: crec[:, s0:s1, k]
                pxs = pxv[:, s0:s1]
                pys = pyv[:, s0:s1]
                # row: 0:x0 1:y0 2:d0 3:d1 4:e0 5:e1 6:det_s 7:detok
                #      8:z0 9:z1 10:z2 11:texbase36
                qx = tt("qx", pxs, ch(0), Alu.subtract)
                qy = tt("qy", pys, ch(1), Alu.subtract)
                t1 = tt("t1", qx[:], ch(5), Alu.mult)
                t2 = tt("t2", qy[:], ch(4), Alu.mult)
                n1 = tt("n1", t1[:], t2[:], Alu.subtract)
                t3 = tt("t3", ch(2), qy[:], Alu.mult)
                t4 = tt("t4", ch(3), qx[:], Alu.mult)
                n2 = tt("n2", t3[:], t4[:], Alu.subtract)
                rdet = p2.tile([128, n], dt.float32, tag=f"rdet{h}")
                nc.vector.reciprocal(rdet[:], ch(6))
                b1 = tt("b1", n1[:], rdet[:], Alu.mult)
                b2 = tt("b2", n2[:], rdet[:], Alu.mult)
                u = ts("u", b1[:], -1.0, 1.0, Alu.mult, Alu.add)
                b0 = tt("b0", u[:], b2[:], Alu.subtract)

                s1v = tt("s1v", n1[:], ch(6), Alu.mult)
                g1 = ts("g1", s1v[:], 0.0, None, Alu.is_ge)
                s2v = tt("s2v", n2[:], ch(6), Alu.mult)
                g2 = ts("g2", s2v[:], 0.0, None, Alu.is_ge)
                g0 = ts("g0", b0[:], 0.0, None, Alu.is_ge)
                m0 = tt("m0", b0[:], ch(8), Alu.mult)
                m1 = tt("m1", b1[:], ch(9), Alu.mult)
                s01 = tt("s01", m0[:], m1[:], Alu.add)
                m2 = tt("m2", b2[:], ch(10), Alu.mult)
                dw = tt("dw", s01[:], m2[:], Alu.add)
                gd = ts("gd", dw[:], 0.0, None, Alu.is_gt)
                vm = tt("vm", g1[:], g2[:], Alu.mult)
                vm = tt("vmb", vm[:], g0[:], Alu.mult)
                vm = tt("vmc", vm[:], gd[:], Alu.mult)
                nc.vector.tensor_tensor(
                    vmb[:, s0:s1], vm[:], ch(7), op=Alu.mult)

                def floor_idx(name, b):
                    x = ts(name + "x", b[:], 6.0, 0.0, Alu.mult, Alu.max)
                    x = ts(name + "c", x[:], 5.5, None, Alu.min)
                    ji = p2.tile([128, n], dt.int32, tag=f"{name}i{h}")
                    nc.vector.tensor_copy(ji[:], x[:])
                    jf = p2.tile([128, n], dt.float32, tag=f"{name}f{h}")
                    nc.vector.tensor_copy(jf[:], ji[:])
                    gt = tt(name + "g", jf[:], x[:], Alu.is_gt)
                    return tt(name + "r", jf[:], gt[:], Alu.subtract)

                i0f = floor_idx("i0", b0)
                i1f = floor_idx("i1", b1)
                i2f = floor_idx("i2", b2)
                if GATHER_MODE == "dma_gather":
                    ffb = p2.tile([128, n], dt.float32, tag=f"ffb{h}")
                    nc.vector.scalar_tensor_tensor(
                        ffb[:], i0f[:], 6.0, ch(12),
                        op0=Alu.mult, op1=Alu.add)
                    ffc = tt("ffc", ffb[:], i1f[:], Alu.add)
                    tidx = shuffle_idxs(f"tx{h}", ffc, s0, s1)
                    tsem, tval = gather(ctex, texg_d, tidx, s0, s1, f"gtex{h}")
                    nc.vector.wait_ge(tsem, tval)
                    # 6-way select on i2: texel k at cols [3k:3k+3]
                    sel = p2.tile([128, n, 3], dt.float32, tag=f"sel{h}")
                    nc.vector.tensor_copy(sel[:], ctex[:, s0:s1, 0:3])
                    for k in range(1, 6):
                        mk = ts(f"mk{k}", i2f[:], float(k), None, Alu.is_equal)
                        nc.vector.copy_predicated(
                            sel[:], mk[:].to_broadcast([128, n, 3]),
                            ctex[:, s0:s1, 3 * k:3 * k + 3])
                else:
                    ffb = p2.tile([128, n], dt.float32, tag=f"ffb{h}")
                    nc.vector.scalar_tensor_tensor(
                        ffb[:], i0f[:], 36.0, ch(11),
                        op0=Alu.mult, op1=Alu.add)
                    ffc = p2.tile([128, n], dt.float32, tag=f"ffc{h}")
                    nc.vector.scalar_tensor_tensor(
                        ffc[:], i1f[:], 6.0, ffb[:],
                        op0=Alu.mult, op1=Alu.add)
                    ffd = tt("ffd", ffc[:], i2f[:], Alu.add)
                    flat = p2.tile([128, n], dt.int32, tag=f"flat{h}")
                    nc.vector.tensor_copy(flat[:], ffd[:])
                    for s in range(s0, s1):
                        nc.gpsimd.indirect_dma_start(
                            out=ctex[:, s, 0:3], out_offset=None,
                            in_=texg_d[:],
                            in_offset=bass.IndirectOffsetOnAxis(
                                ap=flat[:, s - s0:s - s0 + 1], axis=0))
                    sel = ctex[:, s0:s1, 0:3]
                for c in range(3):
                    outp = p2.tile([128, n], dt.float32, tag=f"outp{c}{h}")
                    nc.vector.tensor_tensor(
                        outp[:], sel[:, :, c], vmb[:, s0:s1], op=Alu.mult)
                    nc.sync.dma_start(img_d[c, :, s0:s1], outp[:])

            # ---------------- emission order ------------------------------
            # raster chunk c, then issue chunk c's gathers + phase2 so they
            # overlap chunk c+1's raster
            # non-uniform chunks: light slots are at the end of the slot
            # grid, so shrink the final chunks to minimize the serial tail
            bounds = [0, 8, 16, 24, 32, 40, 48, 56, 60, 62, 64]
            pend = None          # (h, sem) awaiting phase2
            for c in range(len(bounds) - 1):
                s0, s1 = bounds[c], bounds[c + 1]
                mid = (s0 + s1) // 2
                for s in range(s0, mid):
                    if Ks[s] > 0:
                        raster_slot(s)
                    else:
                        nc.gpsimd.memset(i8buf[:, s], 0)
                # feed Pool early: first half-chunk's record gathers
                gather_crec_range(s0, mid)
                for s in range(mid, s1):
                    if Ks[s] > 0:
                        raster_slot(s)
                    else:
                        nc.gpsimd.memset(i8buf[:, s], 0)
                sem_c = gather_crec_range(mid, s1)
                if pend is not None:
                    phase2(pend[0], pend[1])
                pend = ((s0, s1), sem_c)
            phase2(pend[0], pend[1])

    nc.compile()
    return nc


def _get_program(Ks):
    key = ("nc2", GATHER_MODE, tuple(Ks))
    if key not in _CACHE:
        _CACHE[key] = _build_program(Ks)
    return _CACHE[key]


# ----------------------------------------------------------------------------
# Host-side prep
# ----------------------------------------------------------------------------

def _face_setup(vertices, faces, textures):
    f32 = np.float32
    v = np.asarray(vertices[0], f32)
    f = np.asarray(faces[0]).astype(np.int64)
    fv = v[f]

    n = np.cross(fv[:, 1] - fv[:, 0], fv[:, 2] - fv[:, 0]).astype(f32)
    nrm = np.linalg.norm(n, axis=-1, keepdims=True).astype(f32)
    n = (n / (nrm + f32(1e-8))).astype(f32)
    light = (f32(0.5) + f32(0.5) * np.maximum(n[:, 2], f32(0.0))).astype(f32)

    vc = (fv - np.array([0.0, 0.0, -2.0], f32)).astype(f32)
    zc = vc[..., 2].astype(f32)
    wfov = f32(np.tan(np.deg2rad(f32(45.0), dtype=f32), dtype=f32))
    xy = (vc[..., :2] / (zc[..., None] * wfov + f32(1e-8))).astype(f32)

    v0 = xy[:, 0]
    dd = (xy[:, 1] - v0).astype(f32)
    ee = (xy[:, 2] - v0).astype(f32)
    det = (dd[:, 0] * ee[:, 1] - dd[:, 1] * ee[:, 0]).astype(f32)
    det_ok = np.abs(det) > f32(1e-8)
    det_s = np.where(det_ok, det, f32(1.0)).astype(f32)

    x0 = v0[:, 0].astype(np.float64); y0 = v0[:, 1].astype(np.float64)
    d0 = dd[:, 0].astype(np.float64); d1 = dd[:, 1].astype(np.float64)
    e0 = ee[:, 0].astype(np.float64); e1 = ee[:, 1].astype(np.float64)
    ds = det_s.astype(np.float64)
    a1 = np.stack([e1, -e0, e0 * y0 - e1 * x0], -1) / ds[:, None]
    a2 = np.stack([-d1, d0, d1 * x0 - d0 * y0], -1) / ds[:, None]
    a0 = -a1 - a2
    a0[:, 2] += 1.0
    zc64 = zc.astype(np.float64)
    ad = a0 * zc64[:, 0:1] + a1 * zc64[:, 1:2] + a2 * zc64[:, 2:3]

    bad = (~det_ok | ~np.isfinite(a0).all(1) | ~np.isfinite(a1).all(1)
           | ~np.isfinite(a2).all(1) | ~np.isfinite(ad).all(1))
    bad |= np.abs(ad).max(1) > 6e4          # fp16-overflow slivers (none seen)
    for a in (a0, a1, a2):
        mx = np.abs(a).max(1)
        sc = np.where(mx > 3e4, 3e4 / np.maximum(mx, 1e-30), 1.0)
        a *= sc[:, None]

    alive = ~bad & (zc > 0).any(1)

    # NEGATED planes: psum = [-w0|-w1|-w2|-d], relu(+1e34*psum) = penalties
    planes = -np.stack([a0, a1, a2, ad], 1).astype(f32)      # [F, 4, 3]

    frec = np.zeros((F, GREC), f32)
    frec[:, 0] = v0[:, 0]; frec[:, 1] = v0[:, 1]
    frec[:, 2] = dd[:, 0]; frec[:, 3] = dd[:, 1]
    frec[:, 4] = ee[:, 0]; frec[:, 5] = ee[:, 1]
    frec[:, 6] = det_s; frec[:, 7] = det_ok.astype(f32)
    frec[:, 8] = zc[:, 0]; frec[:, 9] = zc[:, 1]; frec[:, 10] = zc[:, 2]
    frec[:, 11] = (np.arange(F) * 216).astype(f32)
    frec[:, 12] = (np.arange(F) * 36).astype(f32)

    tex = np.tanh(np.asarray(textures[0], f32)).astype(f32)  # [F,6,6,6,3]
    texlit = (tex * light[:, None, None, None, None]).astype(f32)
    if GATHER_MODE == "dma_gather":
        texg = np.zeros((TEXROWS + 1, GREC), f32)
        texg[:TEXROWS, :18] = texlit.reshape(TEXROWS, 18)    # [6 i2-texels x 3]
    else:
        texg = texlit.reshape(TEXROWS216, 3).copy()

    xmin = xy[..., 0].min(1); xmax = xy[..., 0].max(1)
    ymin = xy[..., 1].min(1); ymax = xy[..., 1].max(1)
    bbox = np.stack([xmin, xmax, ymin, ymax], -1)

    return planes, frec, texg, bbox, alive


DUMMY_PLANES = -np.array([[0, 0, -1], [0, 0, -1], [0, 0, -1], [0, 0, 1]],
                         np.float32)
DUMMY_FREC = np.zeros(GREC, np.float32)
DUMMY_FREC[6] = 1.0; DUMMY_FREC[8:11] = 1.0


def _schedule(bbox, alive):
    lists = []
    for br in range(32):
        r0 = br * 8
        py_hi = 1 - (r0 + 0.5) / 128
        py_lo = 1 - (r0 + 7.5) / 128
        ysel = alive & (bbox[:, 2] <= py_hi + MARGIN) & (bbox[:, 3] >= py_lo - MARGIN)
        for bc in range(16):
            c0 = bc * 16
            px_lo = (c0 + 0.5) / 128 - 1
            px_hi = (c0 + 15.5) / 128 - 1
            sel = ysel & (bbox[:, 0] <= px_hi + MARGIN) & (bbox[:, 1] >= px_lo - MARGIN)
            lists.append(np.nonzero(sel)[0])
    L = np.array([len(x) for x in lists])
    order = np.argsort(-L, kind="stable")
    # slots in rank order: heaviest first so later (smaller) emission
    # chunks carry the lightest work and the serial tail is minimal
    slot_of_group = list(range(NSLOT))
    Ks = [0] * NSLOT
    assign = np.empty((NSLOT, NCORES), np.int64)
    for g in range(NSLOT):
        s = slot_of_group[g]
        grp = order[g * NCORES:(g + 1) * NCORES]
        assign[s] = grp
        mx = L[grp].max()
        Ks[s] = int(np.ceil(mx / FT)) if mx > 0 else 0
    return tuple(Ks), assign, lists


def _pack_core(core, Ks, assign, lists, planes, frec):
    f32 = np.float32
    npass = int(sum(Ks))
    nrows = npass * FT + 1

    fbh = np.empty((npass, 3, NPLANE), np.float32)
    frs = np.empty((nrows, GREC), f32)
    frs[-1] = DUMMY_FREC
    sbase = np.empty(NSLOT, f32)

    p = 0
    for s in range(NSLOT):
        K = Ks[s]
        if K == 0:
            sbase[s] = npass * FT
            continue
        sbase[s] = p * FT
        fl = lists[assign[s, core]]
        for k in range(K):
            chunk = fl[k * FT:(k + 1) * FT]
            nck = len(chunk)
            blk = np.empty((FT, 4, 3), np.float32)
            blk[:nck] = planes[chunk]
            blk[nck:] = DUMMY_PLANES
            fbh[p] = blk.transpose(2, 1, 0).reshape(3, NPLANE)
            frs[p * FT:p * FT + nck] = frec[chunk]
            frs[p * FT + nck:(p + 1) * FT] = DUMMY_FREC
            p += 1
    assert p == npass

    bh = fbh.astype(np.float16)
    bl = (fbh - bh.astype(np.float32)).astype(np.float16)
    fb = np.concatenate([bh, bl], axis=2)

    pxT = np.empty((3, NSLOT, 128), np.float32)
    pxy = np.empty((2, 128, NSLOT), f32)
    rows = np.empty((128, NSLOT), np.int64)
    cols = np.empty((128, NSLOT), np.int64)
    pj = np.arange(128)
    for s in range(NSLOT):
        b = assign[s, core]
        br, bc = divmod(b, 16)
        r = br * 8 + pj // 16
        c = bc * 16 + pj % 16
        rows[:, s] = r; cols[:, s] = c
        px = ((c + 0.5) / 128.0 - 1.0).astype(f32)
        py = (1.0 - (r + 0.5) / 128.0).astype(f32)
        pxT[0, s] = px; pxT[1, s] = py; pxT[2, s] = 1.0
        pxy[0, :, s] = px; pxy[1, :, s] = py
    pxT = pxT.reshape(3, NSLOT * 128).astype(np.float16)
    sbase_t = np.broadcast_to(sbase, (128, NSLOT)).copy()

    negI = (-np.eye(128, dtype=np.float32))

    return {"pxT": pxT, "pxy": pxy, "fb": fb, "sbase": sbase_t,
            "frec": frs, "negI": negI}, rows, cols


# ----------------------------------------------------------------------------
# Entry point
# ----------------------------------------------------------------------------

def _to_bf16(x):
    import ml_dtypes
    return np.asarray(x, np.float32).astype(ml_dtypes.bfloat16)


def _run(inputs, trace=False):
    from concourse.bass_utils import run_bass_kernel_spmd

    planes, frec, texg, bbox, alive = _face_setup(
        np.asarray(inputs["vertices"]),
        np.asarray(inputs["faces"]),
        np.asarray(inputs["textures"]))
    Ks, assign, lists = _schedule(bbox, alive)
    nc = _get_program(Ks)

    in_maps = []
    maps_rc = []
    for c in range(NCORES):
        m, rows, cols = _pack_core(c, Ks, assign, lists, planes, frec)
        if GATHER_MODE != "dma_gather":
            m["frec"] = np.ascontiguousarray(m["frec"][:, 0:12])
        m["texg"] = texg
        m["negI"] = _to_bf16(m["negI"])
        in_maps.append(m)
        maps_rc.append((rows, cols))
    res = run_bass_kernel_spmd(nc, in_maps, list(range(NCORES)), trace=trace)

    full = np.zeros((3, IMG, IMG), np.float32)
    for c in range(NCORES):
        img = np.asarray(res.results[c]["img"])
        rows, cols = maps_rc[c]
        full[:, rows, cols] = img
    return full[None], res


def kernel(**inputs) -> np.ndarray:
    out, _ = _run(inputs, trace=False)
    return out
